# revision 26
# baseline (speedup 1.0000x reference)
"""Trainium2 Bass kernel for nn_LlamaQuantizedMLP (int4 fake-quant SwiGLU MLP).

Strategy (v4: fully interleaved single-pass stream)
---------------------------------------------------
Reference: per-row int4 fake quant of each weight (scale = max|w|/7,
q = clip(round(w/scale), -8, 7), w' = q*scale), then
  gate = x @ wg'.T ; up = x @ wu'.T ; h = silu(gate)*up ; y = h @ wd'.T

Int4 q values lie in [-8, 7] and are *exactly* representable in fp8 e4m3,
so the weights ship to HBM as fp8 — the ~17 MB/core HBM stream is the
roofline (~425 GB/s/core measured).  The fp8 weight tile is always the
PE-stationary operand (128x128 => compiler-automatic Fast Weight Load,
4 elem/cycle/partition); the tiny activations (8 bf16 columns) move.

v4 changes vs v3 (69.3 us):
 - The down-projection matmuls are *interleaved* into the weight stream
   one i-tile behind gate/up (stream order gu0 gu1 d0 gu2 d1 ... gu10 d9
   d10) instead of running as a serial ~14 us phase after the stream
   drained.  All 352 down matmuls accumulate into ONE open PSUM bank:
   hardware `has_written` bits are per element — the single start=True
   on the first down matmul clears the bank, after which start=False
   matmuls overwrite-on-first-touch / accumulate-on-later-touch per
   element, so 32 output groups can stay open across the whole run.
 - SwiGLU runs per i-tile (3 ops: sigmoid(scale*g) on ACT with the gate
   scale fused in, then two DVE ops, the up scale folded via
   scalar_tensor_tensor), into ping-pong PSUM banks so DVE/ACT never
   touch a bank the PE is writing.
 - The last i-tile is zero-padded 96->128 weight columns so every
   stationary keeps NumWeights==128 (FWL stays enabled; a 96-wide
   stationary triples LDWEIGHTS cost).
 - ~34 dummy matmuls on garbage data run during the dead engine-init
   window (~6.5-10 us, before the first weight block lands) so the PE's
   HAM clock gate is already un-throttled (2.4 GHz) when real matmuls
   start.
 - DMA blocks: small head (PE starts early) and small tail (PE drains
   fast); x first (first matmul needs it), scales after block 0.

Sharding: tensor parallel over the intermediate dim (11008 = 8 x 1376,
zero-padded to 11 tiles of 128 per core).  Each core emits a partial
[4096, 8] output; the host sums the 8 partials and applies down scales.
"""

import numpy as np
import ml_dtypes

import concourse.bacc as bacc
import concourse.mybir as mybir
from concourse.tile import TileContext
from concourse import bass_utils

BF16 = mybir.dt.bfloat16
F32 = mybir.dt.float32
FP8 = mybir.dt.float8e4
U8 = mybir.dt.uint8
NP_BF16 = ml_dtypes.bfloat16
NP_FP8 = ml_dtypes.float8_e4m3

NCORES = 8
WARMUP_MMS = 40


def _block_plan(nw):
    """DMA block sizes (in 16KB weight tiles) summing to nw.

    The HWDGE queue issues roughly one packet (= one partition-row of a
    transfer) per ~26.5 ns, so a transfer's bandwidth is proportional to
    its per-partition row size: 88-tile blocks (11264B rows) saturate
    the ~425 GB/s HBM stream, while e.g. 8-tile blocks crawl at ~40 GB/s.
    Uniform 88s are optimal; smaller head/tail blocks LOSE time."""
    if nw <= 100:
        return [nw]
    # trailing 56+32: the final transfer holds only the PE's last 32
    # tiles, so the post-stream chain after its (engine-64-drain-bound)
    # completion semaphore is just those 32 matmuls
    rem, tail = nw - 88, [56, 32]
    n = (rem + 87) // 88
    base = rem // n
    plan = [base + (1 if i < rem - base * n else 0) for i in range(n)] + tail
    assert sum(plan) == nw
    return plan


class Cfg:
    def __init__(self, b=8, h=4096, i_full=11008, wbufs=4):
        assert h % 128 == 0 and i_full % NCORES == 0
        self.B = b                      # batch = moving-operand columns
        self.H = h
        self.I_FULL = i_full
        self.I_SH = i_full // NCORES    # 1376 per core
        self.IT = (self.I_SH + 127) // 128   # 11 i-tiles
        self.I_PAD = self.IT * 128      # 1408 (both gate/up and down padded)
        self.KC = h // 128              # 32 contraction chunks (gate/up)
        self.HT = h // 128              # 32 output tiles (down)
        self.NGU = self.IT * 2 * self.KC     # 704 gate/up weight tiles
        self.ND = self.IT * self.HT          # 352 down weight tiles
        self.NW = self.NGU + self.ND         # 1056 total (all 128 cols wide)
        self.CW = self.NW * 128              # flat weight columns
        self.PLAN = _block_plan(self.NW)
        assert self.HT * self.B <= 512       # down output fits one PSUM bank
        # x and the scales ride as raw bytes appended to weight block 0
        # (their own transfers would burn 256 packet slots ~= 2.9 MB of
        # stream capacity at the queue's fixed packet rate)
        self.XB = self.KC * self.B * 2       # x bytes/partition (bf16)
        self.SB = 2 * self.IT * self.B * 4   # scale bytes/partition (f32)
        self.XSG = self.XB + self.SB

    # stream/seq indices ----------------------------------------------
    # segment order: gu(0), gu(1), gu(2), dn(0), gu(3), dn(1), ...,
    # gu(IT-1), dn(IT-3), dn(IT-2), dn(IT-1).  Down lags gate/up by TWO
    # i-tiles so the last SwiGLU's ACT->DVE chain hides under dn(IT-3)/
    # dn(IT-2) instead of sitting on the post-stream critical path.
    def gu_base(self, it):
        return it * 2 * self.KC + max(0, it - 2) * self.HT

    def dn_base(self, j):
        if j <= self.IT - 3:
            return (j + 3) * 2 * self.KC + j * self.HT
        return self.IT * 2 * self.KC + j * self.HT


FULL = Cfg()


def build(nc, cfg):
    """Per-core SPMD program (identical on all cores; data differs)."""
    B, IT, KC, HT = cfg.B, cfg.IT, cfg.KC, cfg.HT

    # byte tensor: fp8 weight tiles + x (bf16) + scales (f32) spliced in;
    # uint8 so the simulator's NaN/uninit check ignores the raw bytes
    w_all = nc.dram_tensor("w_all", [128, cfg.CW + cfg.XSG], U8,
                           kind="ExternalInput")
    y2 = nc.dram_tensor("y2", [128, HT * B], F32, kind="ExternalOutput")

    with TileContext(nc) as tc:
        with (
            tc.tile_pool(name="w", bufs=1) as w_pool,
            tc.tile_pool(name="act", bufs=1) as act_pool,
            tc.tile_pool(name="ps", bufs=1, space="PSUM") as ps_pool,
        ):
            # ---- all DMAs ride the sync HWDGE queue (it spreads packets
            # across all 16 SDMA engines; the scalar ring clumps them).
            # Block 0 carries x + scales as extra byte-columns after its
            # 88 tiles, so they arrive with the same 128 packets.
            blocks = []
            seq0 = 0

            def emit_block(bi):
                nonlocal seq0
                nt = cfg.PLAN[bi]
                extra = cfg.XSG if bi == 0 else 0
                c0 = seq0 * 128 + (0 if bi == 0 else cfg.XSG)
                wid = nt * 128 + extra
                wb = w_pool.tile([128, wid], U8, tag=f"wb{bi}",
                                 name=f"wb{bi}")
                # alternate the two HWDGE queues: <=8 transfers per
                # queue avoids Tile's in-FIFO throttle waits (which on
                # the ACT engine would block the SwiGLU sigmoids), and
                # engine-level arbitration lets weight data compete
                # better against instruction-fetch bursts on engine 64
                eng = nc.sync if bi % 2 == 0 else nc.scalar
                eng.dma_start(out=wb[:], in_=w_all[:, c0:c0 + wid])
                blocks.append((seq0 + nt, seq0 * 128, wb))
                seq0 += nt

            for bi in range(len(cfg.PLAN)):
                emit_block(bi)

            def wtile(seq):
                for s1, c0, wb in blocks:
                    if seq < s1:
                        o = seq * 128 - c0
                        return wb[:, o:o + 128].bitcast(FP8)
                raise AssertionError(seq)

            # x / scale views into block 0's extra bytes
            wb0 = blocks[0][2]
            p0 = cfg.PLAN[0] * 128
            x_t = wb0[:, p0:p0 + cfg.XB].bitcast(BF16)        # [128, KC*B]
            sgu_t = wb0[:, p0 + cfg.XB:p0 + cfg.XSG].bitcast(F32)

            # (no PE warm-up matmuls: the PE is semaphore/stream-bound,
            # and every tensor instruction costs IQ-fetch bytes on DMA
            # engine 64 — the stream's critical engine.  Keeping the
            # program just past 8x256 instructions also let the final
            # 16KB IQ refill land AFTER the last weight block, gating
            # the last down matmuls on instruction fetch.)
            h_bf = act_pool.tile([128, IT * B], BF16, tag="hbf")
            # down accumulators split across two banks so the first
            # half's PSUM->SBUF copy + DMA-out can overlap the second
            # half's final matmuls (same bank would serialize: fatal
            # PE-W + DVE-R collision otherwise)
            HT2 = HT // 2
            ps_dnA = ps_pool.tile([128, 512], F32, tag="dnA")
            ps_dnB = ps_pool.tile([128, 512], F32, tag="dnB")

            def emit_dn(j, lo, hi):
                """Down matmuls for i-tile j, output tiles [lo, hi):
                long-lived accumulation groups, one start/stop per bank."""
                base = cfg.dn_base(j)
                for ht in range(lo, hi):
                    ps = ps_dnA if ht < HT2 else ps_dnB
                    c = (ht if ht < HT2 else ht - HT2) * B
                    nc.tensor.matmul(
                        ps[:, c:c + B], wtile(base + ht),
                        h_bf[:, j * B:(j + 1) * B],
                        start=(j == 0 and ht in (0, HT2)),
                        stop=(j == IT - 1 and ht in (HT2 - 1, HT - 1)))

            for it in range(IT):
                if it == IT - 1:
                    # pad the tensor program so the 2048-instruction IQ
                    # block boundary (demand-paged, ~1.5-2.5us fetch
                    # stall on busy engine 64) lands inside gu(IT-1),
                    # where the PE is semaphore-blocked anyway, instead
                    # of in the exposed post-stream burst
                    for _ in range(96):
                        nc.tensor.nop(nofuse=True)
                # ---- gate & up for i-tile it (ping-pong PSUM banks so
                # ACT/DVE read bank parity p while PE fills parity 1-p)
                p = it % 2
                ps_g = ps_pool.tile([128, 512], F32, tag=f"g{p}")
                ps_u = ps_pool.tile([128, 512], F32, tag=f"u{p}")
                base = cfg.gu_base(it)
                for gu, ps in ((0, ps_g), (1, ps_u)):
                    for k in range(KC):
                        nc.tensor.matmul(
                            ps[:, 0:B], wtile(base + gu * KC + k),
                            x_t[:, k * B:(k + 1) * B],
                            start=(k == 0), stop=(k == KC - 1))
                # ---- SwiGLU for i-tile it:
                #   h = G*sigmoid(sg*G) * U*(sg*su)
                # with G/U the raw psum gate/up and sg/su the quant
                # scales (pad rows have zero weights => G=U=0 => h=0).
                sg_col = sgu_t[:, it * B:it * B + 1]
                sp_col = sgu_t[:, (IT + it) * B:(IT + it) * B + 1]
                sig = act_pool.tile([128, B], F32, tag=f"sig{p}")
                nc.scalar.activation(
                    out=sig[:], in_=ps_g[:, 0:B],
                    func=mybir.ActivationFunctionType.Sigmoid, scale=sg_col)
                a_sb = act_pool.tile([128, B], F32, tag=f"a{p}")
                nc.vector.tensor_mul(out=a_sb[:], in0=ps_g[:, 0:B],
                                     in1=sig[:])
                nc.vector.scalar_tensor_tensor(
                    out=h_bf[:, it * B:(it + 1) * B], in0=ps_u[:, 0:B],
                    scalar=sp_col, in1=a_sb[:],
                    op0=mybir.AluOpType.mult, op1=mybir.AluOpType.mult)
                # ---- down matmuls, two i-tiles behind (keeps the PE
                # consuming the stream in DMA arrival order)
                if it >= 2:
                    emit_dn(it - 2, 0, HT)
            # last two i-tiles; flush the A half while B's matmuls run
            y_sb = act_pool.tile([128, HT * B], F32, tag="ysb")
            if IT >= 2:
                emit_dn(IT - 2, 0, HT)
            emit_dn(IT - 1, 0, HT2)
            nc.vector.tensor_copy(out=y_sb[:, 0:HT2 * B],
                                  in_=ps_dnA[:, 0:HT2 * B])
            nc.sync.dma_start(out=y2[:, 0:HT2 * B], in_=y_sb[:, 0:HT2 * B])
            emit_dn(IT - 1, HT2, HT)
            nc.vector.tensor_copy(out=y_sb[:, HT2 * B:HT * B],
                                  in_=ps_dnB[:, 0:HT2 * B])
            nc.sync.dma_start(out=y2[:, HT2 * B:HT * B],
                              in_=y_sb[:, HT2 * B:HT * B])

    nc.compile()
    return nc


# ---------------------------------------------------------------------------
# host-side preparation
# ---------------------------------------------------------------------------

def _quant(w):
    """Reference int4 fake-quant: integer q (f32) and per-row scale."""
    w = np.asarray(w, np.float32)
    scale = (np.max(np.abs(w), axis=1, keepdims=True) /
             np.float32(7.0)).astype(np.float32)
    scale = np.maximum(scale, np.float32(np.finfo(np.float32).tiny))
    q = np.clip(np.round((w / scale).astype(np.float32)), -8.0, 7.0).astype(
        np.float32)
    return q, scale


def make_in_maps(x, w_gate, w_up, w_down, cfg):
    """Returns (in_maps for 8 cores, down-scale vector [H])."""
    B, H, IT, KC, HT = cfg.B, cfg.H, cfg.IT, cfg.KC, cfg.HT
    qg, sgf = _quant(w_gate)
    qu, suf = _quant(w_up)
    qd, sdf = _quant(w_down)

    # x: [B,1,H] f32 -> [128, KC, B] bf16  ([r,k,b] = x[b, k*128+r])
    x2 = np.asarray(x, np.float32).reshape(B, H)
    xt = np.ascontiguousarray(
        x2.T.reshape(KC, 128, B).transpose(1, 0, 2).astype(NP_BF16))

    in_maps = []
    for c in range(NCORES):
        isl = slice(c * cfg.I_SH, (c + 1) * cfg.I_SH)
        qg_sh = np.zeros((cfg.I_PAD, H), np.float32)
        qg_sh[0:cfg.I_SH] = qg[isl]
        qu_sh = np.zeros((cfg.I_PAD, H), np.float32)
        qu_sh[0:cfg.I_SH] = qu[isl]
        qd_sh = np.zeros((H, cfg.I_PAD), np.float32)
        qd_sh[:, 0:cfg.I_SH] = qd[:, isl]

        # flat-column layout in exact PE/stream consumption order:
        # gu(0), gu(1), dn(0), gu(2), dn(1), ..., gu(IT-1), dn(IT-2),
        # dn(IT-1).  gate/up tile (it,k) = [128 h-part, 128 i-cols];
        # down tile (j,ht) = [128 i-part, 128 h-cols].
        def gu_seg(it):
            out = []
            for q_sh in (qg_sh, qu_sh):
                qq = q_sh[it * 128:(it + 1) * 128]       # [128 i, H]
                out.append(qq.T.reshape(KC, 128, 128).transpose(1, 0, 2)
                           .reshape(128, KC * 128))
            return out

        def dn_seg(j):
            wd = qd_sh[:, j * 128:(j + 1) * 128].T       # [128 i, H]
            return [wd.reshape(128, HT * 128)]

        cols = []
        for it in range(IT):
            cols += gu_seg(it)
            if it >= 2:
                cols += dn_seg(it - 2)
        if IT >= 2:
            cols += dn_seg(IT - 2)
        cols += dn_seg(IT - 1)
        w_tiles = np.concatenate(cols, axis=1).astype(NP_FP8)
        assert w_tiles.shape[1] == cfg.CW

        # per-row scales, broadcast over batch: [r, it*B+b] = s[it*128+r]
        def sc_b(s_col):               # [I_SH] -> [128, IT*B] f32
            s_pad = np.zeros((cfg.I_PAD,), np.float32)
            s_pad[0:cfg.I_SH] = s_col
            return np.ascontiguousarray(np.broadcast_to(
                s_pad.reshape(IT, 128, 1).transpose(1, 0, 2),
                (128, IT, B)).reshape(128, IT * B))

        sg_sh = sgf[isl, 0]
        sp_sh = sgf[isl, 0] * suf[isl, 0]
        sgu_np = np.ascontiguousarray(np.concatenate(
            [sc_b(sg_sh), sc_b(sp_sh)], axis=1).astype(np.float32))

        # splice x + scales as raw bytes after block 0's tiles
        p0 = cfg.PLAN[0] * 128
        w_u8 = w_tiles.view(np.uint8)
        x_u8 = np.ascontiguousarray(xt.reshape(128, -1)).view(np.uint8)
        s_u8 = sgu_np.view(np.uint8)
        w_all = np.ascontiguousarray(np.concatenate(
            [w_u8[:, :p0], x_u8, s_u8, w_u8[:, p0:]], axis=1))
        assert w_all.shape[1] == cfg.CW + cfg.XSG

        in_maps.append({"w_all": w_all})
    return in_maps, sdf[:, 0]


_NC_CACHE = {}


def _get_nc(cfg):
    key = (cfg.B, cfg.H, cfg.I_FULL)
    if key not in _NC_CACHE:
        nc = bacc.Bacc(None, target_bir_lowering=False)
        build(nc, cfg)
        _NC_CACHE[key] = nc
    return _NC_CACHE[key]


def run(x, w_gate, w_up, w_down, cfg=FULL, **spmd_kwargs):
    """Full pipeline; returns (output [B,1,H] f32, BassKernelResults)."""
    in_maps, sd = make_in_maps(x, w_gate, w_up, w_down, cfg)
    nc = _get_nc(cfg)
    res = bass_utils.run_bass_kernel_spmd(
        nc, in_maps, core_ids=list(range(NCORES)), **spmd_kwargs)
    acc = np.zeros((128, cfg.HT * cfg.B), np.float32)
    for r in res.results:
        acc += r["y2"]
    # y2 [r, ht*B+b] = partial y[b, ht*128+r]
    y = acc.reshape(128, cfg.HT, cfg.B).transpose(2, 1, 0).reshape(
        cfg.B, cfg.H)
    y = y * sd[None, :]
    return y.reshape(cfg.B, 1, cfg.H).astype(np.float32), res


def kernel(x, w_gate, w_up, w_down):
    out, _ = run(x, w_gate, w_up, w_down)
    return out


# revision 27
# speedup vs baseline: 1.0860x; 1.0860x over previous
"""Trainium2 Bass kernel for nn_LlamaQuantizedMLP (int4 fake-quant SwiGLU MLP).

Strategy (v4: fully interleaved single-pass stream)
---------------------------------------------------
Reference: per-row int4 fake quant of each weight (scale = max|w|/7,
q = clip(round(w/scale), -8, 7), w' = q*scale), then
  gate = x @ wg'.T ; up = x @ wu'.T ; h = silu(gate)*up ; y = h @ wd'.T

Int4 q values lie in [-8, 7] and are *exactly* representable in fp8 e4m3,
so the weights ship to HBM as fp8 — the ~17 MB/core HBM stream is the
roofline (~425 GB/s/core measured).  The fp8 weight tile is always the
PE-stationary operand (128x128 => compiler-automatic Fast Weight Load,
4 elem/cycle/partition); the tiny activations (8 bf16 columns) move.

v4 changes vs v3 (69.3 us):
 - The down-projection matmuls are *interleaved* into the weight stream
   one i-tile behind gate/up (stream order gu0 gu1 d0 gu2 d1 ... gu10 d9
   d10) instead of running as a serial ~14 us phase after the stream
   drained.  All 352 down matmuls accumulate into ONE open PSUM bank:
   hardware `has_written` bits are per element — the single start=True
   on the first down matmul clears the bank, after which start=False
   matmuls overwrite-on-first-touch / accumulate-on-later-touch per
   element, so 32 output groups can stay open across the whole run.
 - SwiGLU runs per i-tile (3 ops: sigmoid(scale*g) on ACT with the gate
   scale fused in, then two DVE ops, the up scale folded via
   scalar_tensor_tensor), into ping-pong PSUM banks so DVE/ACT never
   touch a bank the PE is writing.
 - The last i-tile is zero-padded 96->128 weight columns so every
   stationary keeps NumWeights==128 (FWL stays enabled; a 96-wide
   stationary triples LDWEIGHTS cost).
 - ~34 dummy matmuls on garbage data run during the dead engine-init
   window (~6.5-10 us, before the first weight block lands) so the PE's
   HAM clock gate is already un-throttled (2.4 GHz) when real matmuls
   start.
 - DMA blocks: small head (PE starts early) and small tail (PE drains
   fast); x first (first matmul needs it), scales after block 0.

Sharding: tensor parallel over the intermediate dim (11008 = 8 x 1376,
zero-padded to 11 tiles of 128 per core).  Each core emits a partial
[4096, 8] output; the host sums the 8 partials and applies down scales.
"""

import numpy as np
import ml_dtypes

import concourse.bacc as bacc
import concourse.mybir as mybir
from concourse.tile import TileContext
from concourse import bass_utils

BF16 = mybir.dt.bfloat16
F32 = mybir.dt.float32
FP8 = mybir.dt.float8e4
U8 = mybir.dt.uint8
NP_BF16 = ml_dtypes.bfloat16
NP_FP8 = ml_dtypes.float8_e4m3

NCORES = 8
WARMUP_MMS = 40


def _block_plan(nw):
    """DMA block sizes (in 16KB weight tiles) summing to nw.

    The HWDGE queue issues roughly one packet (= one partition-row of a
    transfer) per ~26.5 ns, so a transfer's bandwidth is proportional to
    its per-partition row size: 88-tile blocks (11264B rows) saturate
    the ~425 GB/s HBM stream, while e.g. 8-tile blocks crawl at ~40 GB/s.
    Uniform 88s are optimal; smaller head/tail blocks LOSE time."""
    if nw <= 100:
        return [nw]
    # trailing 56+32: the final transfer holds only the PE's last 32
    # tiles, so the post-stream chain after its (engine-64-drain-bound)
    # completion semaphore is just those 32 matmuls
    rem, tail = nw - 88, [56, 32]
    n = (rem + 87) // 88
    base = rem // n
    plan = [base + (1 if i < rem - base * n else 0) for i in range(n)] + tail
    assert sum(plan) == nw
    return plan


class Cfg:
    def __init__(self, b=8, h=4096, i_full=11008, wbufs=4):
        assert h % 128 == 0 and i_full % NCORES == 0
        self.B = b                      # batch = moving-operand columns
        self.H = h
        self.I_FULL = i_full
        self.I_SH = i_full // NCORES    # 1376 per core
        self.IT = (self.I_SH + 127) // 128   # 11 i-tiles
        self.I_PAD = self.IT * 128      # 1408 (both gate/up and down padded)
        self.KC = h // 128              # 32 contraction chunks (gate/up)
        self.HT = h // 128              # 32 output tiles (down)
        self.NGU = self.IT * 2 * self.KC     # 704 gate/up weight tiles
        self.ND = self.IT * self.HT          # 352 down weight tiles
        self.NW = self.NGU + self.ND         # 1056 total (all 128 cols wide)
        self.CW = self.NW * 128              # flat weight columns
        self.PLAN = _block_plan(self.NW)
        assert self.HT * self.B <= 512       # down output fits one PSUM bank
        # x and the scales ride as raw bytes appended to weight block 0
        # (their own transfers would burn 256 packet slots ~= 2.9 MB of
        # stream capacity at the queue's fixed packet rate)
        self.XB = self.KC * self.B * 2       # x bytes/partition (bf16)
        self.SB = 2 * self.IT * self.B * 4   # scale bytes/partition (f32)
        self.XSG = self.XB + self.SB

    # stream/seq indices ----------------------------------------------
    # segment order: gu(0), gu(1), gu(2), dn(0), gu(3), dn(1), ...,
    # gu(IT-1), dn(IT-3), dn(IT-2), dn(IT-1).  Down lags gate/up by TWO
    # i-tiles so the last SwiGLU's ACT->DVE chain hides under dn(IT-3)/
    # dn(IT-2) instead of sitting on the post-stream critical path.
    def gu_base(self, it):
        return it * 2 * self.KC + max(0, it - 2) * self.HT

    def dn_base(self, j):
        if j <= self.IT - 3:
            return (j + 3) * 2 * self.KC + j * self.HT
        return self.IT * 2 * self.KC + j * self.HT


FULL = Cfg()


def build(nc, cfg):
    """Per-core SPMD program (identical on all cores; data differs)."""
    B, IT, KC, HT = cfg.B, cfg.IT, cfg.KC, cfg.HT

    # byte tensor: fp8 weight tiles + x (bf16) + scales (f32) spliced in;
    # uint8 so the simulator's NaN/uninit check ignores the raw bytes
    w_all = nc.dram_tensor("w_all", [128, cfg.CW + cfg.XSG], U8,
                           kind="ExternalInput")
    y2 = nc.dram_tensor("y2", [128, HT * B], F32, kind="ExternalOutput")

    with TileContext(nc) as tc:
        with (
            tc.tile_pool(name="w", bufs=1) as w_pool,
            tc.tile_pool(name="act", bufs=1) as act_pool,
            tc.tile_pool(name="ps", bufs=1, space="PSUM") as ps_pool,
        ):
            # ---- all DMAs ride the sync HWDGE queue (it spreads packets
            # across all 16 SDMA engines; the scalar ring clumps them).
            # Block 0 carries x + scales as extra byte-columns after its
            # 88 tiles, so they arrive with the same 128 packets.
            blocks = []
            seq0 = 0

            def emit_block(bi):
                nonlocal seq0
                nt = cfg.PLAN[bi]
                extra = cfg.XSG if bi == 0 else 0
                c0 = seq0 * 128 + (0 if bi == 0 else cfg.XSG)
                wid = nt * 128 + extra
                wb = w_pool.tile([128, wid], U8, tag=f"wb{bi}",
                                 name=f"wb{bi}")
                nc.sync.dma_start(out=wb[:], in_=w_all[:, c0:c0 + wid])
                blocks.append((seq0 + nt, seq0 * 128, wb))
                seq0 += nt

            for bi in range(len(cfg.PLAN)):
                emit_block(bi)

            def wtile(seq):
                for s1, c0, wb in blocks:
                    if seq < s1:
                        o = seq * 128 - c0
                        return wb[:, o:o + 128].bitcast(FP8)
                raise AssertionError(seq)

            # x / scale views into block 0's extra bytes
            wb0 = blocks[0][2]
            p0 = cfg.PLAN[0] * 128
            x_t = wb0[:, p0:p0 + cfg.XB].bitcast(BF16)        # [128, KC*B]
            sgu_t = wb0[:, p0 + cfg.XB:p0 + cfg.XSG].bitcast(F32)

            # (no PE warm-up matmuls: the PE is semaphore/stream-bound,
            # and every tensor instruction costs IQ-fetch bytes on DMA
            # engine 64 — the stream's critical engine.  Keeping the
            # program just past 8x256 instructions also let the final
            # 16KB IQ refill land AFTER the last weight block, gating
            # the last down matmuls on instruction fetch.)
            h_bf = act_pool.tile([128, IT * B], BF16, tag="hbf")
            # down accumulators split across two banks so the first
            # half's PSUM->SBUF copy + DMA-out can overlap the second
            # half's final matmuls (same bank would serialize: fatal
            # PE-W + DVE-R collision otherwise)
            HT2 = HT // 2
            ps_dnA = ps_pool.tile([128, 512], F32, tag="dnA")
            ps_dnB = ps_pool.tile([128, 512], F32, tag="dnB")

            def emit_dn(j, lo, hi):
                """Down matmuls for i-tile j, output tiles [lo, hi):
                long-lived accumulation groups, one start/stop per bank."""
                base = cfg.dn_base(j)
                for ht in range(lo, hi):
                    ps = ps_dnA if ht < HT2 else ps_dnB
                    c = (ht if ht < HT2 else ht - HT2) * B
                    nc.tensor.matmul(
                        ps[:, c:c + B], wtile(base + ht),
                        h_bf[:, j * B:(j + 1) * B],
                        start=(j == 0 and ht in (0, HT2)),
                        stop=(j == IT - 1 and ht in (HT2 - 1, HT - 1)))

            for it in range(IT):
                if it == IT - 1:
                    # pad the tensor program so the 2048-instruction IQ
                    # block boundary (demand-paged, ~1.5-2.5us fetch
                    # stall on busy engine 64) lands inside gu(IT-1),
                    # where the PE is semaphore-blocked anyway, instead
                    # of in the exposed post-stream burst
                    for _ in range(96):
                        nc.tensor.nop(nofuse=True)
                # ---- gate & up for i-tile it (ping-pong PSUM banks so
                # ACT/DVE read bank parity p while PE fills parity 1-p)
                p = it % 2
                ps_g = ps_pool.tile([128, 512], F32, tag=f"g{p}")
                ps_u = ps_pool.tile([128, 512], F32, tag=f"u{p}")
                base = cfg.gu_base(it)
                for gu, ps in ((0, ps_g), (1, ps_u)):
                    for k in range(KC):
                        nc.tensor.matmul(
                            ps[:, 0:B], wtile(base + gu * KC + k),
                            x_t[:, k * B:(k + 1) * B],
                            start=(k == 0), stop=(k == KC - 1))
                # ---- SwiGLU for i-tile it:
                #   h = G*sigmoid(sg*G) * U*(sg*su)
                # with G/U the raw psum gate/up and sg/su the quant
                # scales (pad rows have zero weights => G=U=0 => h=0).
                sg_col = sgu_t[:, it * B:it * B + 1]
                sp_col = sgu_t[:, (IT + it) * B:(IT + it) * B + 1]
                sig = act_pool.tile([128, B], F32, tag=f"sig{p}")
                nc.scalar.activation(
                    out=sig[:], in_=ps_g[:, 0:B],
                    func=mybir.ActivationFunctionType.Sigmoid, scale=sg_col)
                a_sb = act_pool.tile([128, B], F32, tag=f"a{p}")
                nc.vector.tensor_mul(out=a_sb[:], in0=ps_g[:, 0:B],
                                     in1=sig[:])
                nc.vector.scalar_tensor_tensor(
                    out=h_bf[:, it * B:(it + 1) * B], in0=ps_u[:, 0:B],
                    scalar=sp_col, in1=a_sb[:],
                    op0=mybir.AluOpType.mult, op1=mybir.AluOpType.mult)
                # ---- down matmuls, two i-tiles behind (keeps the PE
                # consuming the stream in DMA arrival order)
                if it >= 2:
                    emit_dn(it - 2, 0, HT)
            # last two i-tiles; flush the A half while B's matmuls run
            y_sb = act_pool.tile([128, HT * B], F32, tag="ysb")
            if IT >= 2:
                emit_dn(IT - 2, 0, HT)
            emit_dn(IT - 1, 0, HT2)
            nc.vector.tensor_copy(out=y_sb[:, 0:HT2 * B],
                                  in_=ps_dnA[:, 0:HT2 * B])
            nc.sync.dma_start(out=y2[:, 0:HT2 * B], in_=y_sb[:, 0:HT2 * B])
            emit_dn(IT - 1, HT2, HT)
            nc.vector.tensor_copy(out=y_sb[:, HT2 * B:HT * B],
                                  in_=ps_dnB[:, 0:HT2 * B])
            nc.sync.dma_start(out=y2[:, HT2 * B:HT * B],
                              in_=y_sb[:, HT2 * B:HT * B])

    nc.compile()
    return nc


# ---------------------------------------------------------------------------
# host-side preparation
# ---------------------------------------------------------------------------

def _quant(w):
    """Reference int4 fake-quant: integer q (f32) and per-row scale."""
    w = np.asarray(w, np.float32)
    scale = (np.max(np.abs(w), axis=1, keepdims=True) /
             np.float32(7.0)).astype(np.float32)
    scale = np.maximum(scale, np.float32(np.finfo(np.float32).tiny))
    q = np.clip(np.round((w / scale).astype(np.float32)), -8.0, 7.0).astype(
        np.float32)
    return q, scale


def make_in_maps(x, w_gate, w_up, w_down, cfg):
    """Returns (in_maps for 8 cores, down-scale vector [H])."""
    B, H, IT, KC, HT = cfg.B, cfg.H, cfg.IT, cfg.KC, cfg.HT
    qg, sgf = _quant(w_gate)
    qu, suf = _quant(w_up)
    qd, sdf = _quant(w_down)

    # x: [B,1,H] f32 -> [128, KC, B] bf16  ([r,k,b] = x[b, k*128+r])
    x2 = np.asarray(x, np.float32).reshape(B, H)
    xt = np.ascontiguousarray(
        x2.T.reshape(KC, 128, B).transpose(1, 0, 2).astype(NP_BF16))

    in_maps = []
    for c in range(NCORES):
        isl = slice(c * cfg.I_SH, (c + 1) * cfg.I_SH)
        qg_sh = np.zeros((cfg.I_PAD, H), np.float32)
        qg_sh[0:cfg.I_SH] = qg[isl]
        qu_sh = np.zeros((cfg.I_PAD, H), np.float32)
        qu_sh[0:cfg.I_SH] = qu[isl]
        qd_sh = np.zeros((H, cfg.I_PAD), np.float32)
        qd_sh[:, 0:cfg.I_SH] = qd[:, isl]

        # flat-column layout in exact PE/stream consumption order:
        # gu(0), gu(1), dn(0), gu(2), dn(1), ..., gu(IT-1), dn(IT-2),
        # dn(IT-1).  gate/up tile (it,k) = [128 h-part, 128 i-cols];
        # down tile (j,ht) = [128 i-part, 128 h-cols].
        def gu_seg(it):
            out = []
            for q_sh in (qg_sh, qu_sh):
                qq = q_sh[it * 128:(it + 1) * 128]       # [128 i, H]
                out.append(qq.T.reshape(KC, 128, 128).transpose(1, 0, 2)
                           .reshape(128, KC * 128))
            return out

        def dn_seg(j):
            wd = qd_sh[:, j * 128:(j + 1) * 128].T       # [128 i, H]
            return [wd.reshape(128, HT * 128)]

        cols = []
        for it in range(IT):
            cols += gu_seg(it)
            if it >= 2:
                cols += dn_seg(it - 2)
        if IT >= 2:
            cols += dn_seg(IT - 2)
        cols += dn_seg(IT - 1)
        w_tiles = np.concatenate(cols, axis=1).astype(NP_FP8)
        assert w_tiles.shape[1] == cfg.CW

        # per-row scales, broadcast over batch: [r, it*B+b] = s[it*128+r]
        def sc_b(s_col):               # [I_SH] -> [128, IT*B] f32
            s_pad = np.zeros((cfg.I_PAD,), np.float32)
            s_pad[0:cfg.I_SH] = s_col
            return np.ascontiguousarray(np.broadcast_to(
                s_pad.reshape(IT, 128, 1).transpose(1, 0, 2),
                (128, IT, B)).reshape(128, IT * B))

        sg_sh = sgf[isl, 0]
        sp_sh = sgf[isl, 0] * suf[isl, 0]
        sgu_np = np.ascontiguousarray(np.concatenate(
            [sc_b(sg_sh), sc_b(sp_sh)], axis=1).astype(np.float32))

        # splice x + scales as raw bytes after block 0's tiles
        p0 = cfg.PLAN[0] * 128
        w_u8 = w_tiles.view(np.uint8)
        x_u8 = np.ascontiguousarray(xt.reshape(128, -1)).view(np.uint8)
        s_u8 = sgu_np.view(np.uint8)
        w_all = np.ascontiguousarray(np.concatenate(
            [w_u8[:, :p0], x_u8, s_u8, w_u8[:, p0:]], axis=1))
        assert w_all.shape[1] == cfg.CW + cfg.XSG

        in_maps.append({"w_all": w_all})
    return in_maps, sdf[:, 0]


_NC_CACHE = {}


def _get_nc(cfg):
    key = (cfg.B, cfg.H, cfg.I_FULL)
    if key not in _NC_CACHE:
        nc = bacc.Bacc(None, target_bir_lowering=False)
        build(nc, cfg)
        _NC_CACHE[key] = nc
    return _NC_CACHE[key]


def run(x, w_gate, w_up, w_down, cfg=FULL, **spmd_kwargs):
    """Full pipeline; returns (output [B,1,H] f32, BassKernelResults)."""
    in_maps, sd = make_in_maps(x, w_gate, w_up, w_down, cfg)
    nc = _get_nc(cfg)
    res = bass_utils.run_bass_kernel_spmd(
        nc, in_maps, core_ids=list(range(NCORES)), **spmd_kwargs)
    acc = np.zeros((128, cfg.HT * cfg.B), np.float32)
    for r in res.results:
        acc += r["y2"]
    # y2 [r, ht*B+b] = partial y[b, ht*128+r]
    y = acc.reshape(128, cfg.HT, cfg.B).transpose(2, 1, 0).reshape(
        cfg.B, cfg.H)
    y = y * sd[None, :]
    return y.reshape(cfg.B, 1, cfg.H).astype(np.float32), res


def kernel(x, w_gate, w_up, w_down):
    out, _ = run(x, w_gate, w_up, w_down)
    return out


# revision 30
# speedup vs baseline: 1.1072x; 1.0196x over previous
"""Trainium2 Bass kernel for nn_LlamaQuantizedMLP (int4 fake-quant SwiGLU MLP).

Strategy (v9: fully interleaved single-pass stream)
---------------------------------------------------
Reference: per-row int4 fake quant of each weight (scale = max|w|/7,
q = clip(round(w/scale), -8, 7), w' = q*scale), then
  gate = x @ wg'.T ; up = x @ wu'.T ; h = silu(gate)*up ; y = h @ wd'.T

Int4 q values lie in [-8, 7] and are *exactly* representable in fp8 e4m3,
so the weights ship to HBM as fp8 — the ~17 MB/core HBM stream is the
roofline (~425 GB/s/core measured).  The fp8 weight tile is always the
PE-stationary operand (128x128 => compiler-automatic Fast Weight Load);
the tiny activations (8 bf16 columns) move.

Design points (69.3 us baseline -> ~68 us; the run is bound by DMA
engine 64, which carries 1/16 of the weight stream PLUS the ~140KB
tensor-engine instruction stream, so most "obvious" wins don't move):
 - ONE pass: down-projection matmuls are interleaved into the weight
   stream two i-tiles behind gate/up (gu0 gu1 gu2 d0 gu3 d1 ... gu10
   d8 d9 d10) instead of a serial ~14us phase after the stream drains.
   All 352 down matmuls accumulate into two long-lived PSUM banks
   (A: output tiles 0-15, B: 16-31): `has_written` bits are per
   element, so one start=True per bank opens it and 11x16 groups stay
   open across the whole run; the A-half flush overlaps B's last MMs.
 - SwiGLU runs per i-tile (sigmoid(scale*g) on ACT with the gate scale
   fused in, then two DVE ops with the up scale folded via
   scalar_tensor_tensor), in ping-pong PSUM banks so DVE/ACT never
   touch a bank the PE is writing.
 - The HWDGE queue issues ~1 packet (= one partition-row) per ~26.5ns,
   so only large per-partition rows reach line rate: uniform 88-tile
   (11264B-row) blocks, with a 56+32 tail so the post-stream chain
   after the last (engine-64-drain-bound) semaphore is just 32 MMs.
 - x + scales ride as raw bytes appended to weight block 0 (bitcast
   views) — separate transfers would burn 256 packet slots (~2.9MB of
   stream capacity).
 - Everything padded to 128-wide stationaries (FWL needs NumWeights
   == 128; a 96-wide stationary triples LDWEIGHTS cost).
 - 96 tensor nops before gu(IT-1) place the demand-paged 2048-instr
   IQ-fetch boundary where the PE is semaphore-blocked anyway.
 - Measured dead ends: moving any weight blocks to the scalar/ACT
   HWDGE queue costs +7us (dma_starts block SwiGLU sigmoids in the
   ACT FIFO, and it relieves nothing: engine 64's load is invariant);
   PE warmup matmuls cost more in instruction-fetch bytes than the
   HAM clock-gate ramp they save.

Sharding: tensor parallel over the intermediate dim (11008 = 8 x 1376,
zero-padded to 11 tiles of 128 per core).  Each core emits a partial
[4096, 8] output; the host sums the 8 partials and applies down scales.
"""

import numpy as np
import ml_dtypes

import concourse.bacc as bacc
import concourse.mybir as mybir
from concourse.tile import TileContext
from concourse import bass_utils

BF16 = mybir.dt.bfloat16
F32 = mybir.dt.float32
FP8 = mybir.dt.float8e4
U8 = mybir.dt.uint8
NP_BF16 = ml_dtypes.bfloat16
NP_FP8 = ml_dtypes.float8_e4m3

NCORES = 8


def _block_plan(nw):
    """DMA block sizes (in 16KB weight tiles) summing to nw.

    The HWDGE queue issues roughly one packet (= one partition-row of a
    transfer) per ~26.5 ns, so a transfer's bandwidth is proportional to
    its per-partition row size: 88-tile blocks (11264B rows) saturate
    the ~425 GB/s HBM stream, while e.g. 8-tile blocks crawl at ~40 GB/s.
    Uniform 88s are optimal; smaller head/tail blocks LOSE time."""
    if nw <= 100:
        return [nw]
    # trailing 56+32: the final transfer holds only the PE's last 32
    # tiles, so the post-stream chain after its (engine-64-drain-bound)
    # completion semaphore is just those 32 matmuls
    rem, tail = nw - 88, [56, 32]
    n = (rem + 87) // 88
    base = rem // n
    plan = [base + (1 if i < rem - base * n else 0) for i in range(n)] + tail
    assert sum(plan) == nw
    return plan


class Cfg:
    def __init__(self, b=8, h=4096, i_full=11008, wbufs=4):
        assert h % 128 == 0 and i_full % NCORES == 0
        self.B = b                      # batch = moving-operand columns
        self.H = h
        self.I_FULL = i_full
        self.I_SH = i_full // NCORES    # 1376 per core
        self.IT = (self.I_SH + 127) // 128   # 11 i-tiles
        self.I_PAD = self.IT * 128      # 1408 (both gate/up and down padded)
        self.KC = h // 128              # 32 contraction chunks (gate/up)
        self.HT = h // 128              # 32 output tiles (down)
        self.NGU = self.IT * 2 * self.KC     # 704 gate/up weight tiles
        self.ND = self.IT * self.HT          # 352 down weight tiles
        self.NW = self.NGU + self.ND         # 1056 total (all 128 cols wide)
        self.CW = self.NW * 128              # flat weight columns
        self.PLAN = _block_plan(self.NW)
        assert self.HT * self.B <= 512       # down output fits one PSUM bank
        # x and the scales ride as raw bytes appended to weight block 0
        # (their own transfers would burn 256 packet slots ~= 2.9 MB of
        # stream capacity at the queue's fixed packet rate)
        self.XB = self.KC * self.B * 2       # x bytes/partition (bf16)
        self.SB = 2 * self.IT * self.B * 4   # scale bytes/partition (f32)
        self.XSG = self.XB + self.SB

    # stream/seq indices ----------------------------------------------
    # segment order: gu(0), gu(1), gu(2), dn(0), gu(3), dn(1), ...,
    # gu(IT-1), dn(IT-3), dn(IT-2), dn(IT-1).  Down lags gate/up by TWO
    # i-tiles so the last SwiGLU's ACT->DVE chain hides under dn(IT-3)/
    # dn(IT-2) instead of sitting on the post-stream critical path.
    def gu_base(self, it):
        return it * 2 * self.KC + max(0, it - 2) * self.HT

    def dn_base(self, j):
        if j <= self.IT - 3:
            return (j + 3) * 2 * self.KC + j * self.HT
        return self.IT * 2 * self.KC + j * self.HT


FULL = Cfg()


def build(nc, cfg):
    """Per-core SPMD program (identical on all cores; data differs)."""
    B, IT, KC, HT = cfg.B, cfg.IT, cfg.KC, cfg.HT

    # byte tensor: fp8 weight tiles + x (bf16) + scales (f32) spliced in;
    # uint8 so the simulator's NaN/uninit check ignores the raw bytes
    w_all = nc.dram_tensor("w_all", [128, cfg.CW + cfg.XSG], U8,
                           kind="ExternalInput")
    y2 = nc.dram_tensor("y2", [128, HT * B], F32, kind="ExternalOutput")

    with TileContext(nc) as tc:
        with (
            tc.tile_pool(name="w", bufs=1) as w_pool,
            tc.tile_pool(name="act", bufs=1) as act_pool,
            tc.tile_pool(name="ps", bufs=1, space="PSUM") as ps_pool,
        ):
            # ---- all DMAs ride the sync HWDGE queue (the sync engine
            # has no compute to block; see docstring for why the
            # scalar/ACT queue loses).  Block 0 carries x + scales as
            # extra byte-columns after its 88 tiles, so they arrive
            # with the same 128 packets.
            blocks = []
            seq0 = 0

            def emit_block(bi):
                nonlocal seq0
                nt = cfg.PLAN[bi]
                extra = cfg.XSG if bi == 0 else 0
                c0 = seq0 * 128 + (0 if bi == 0 else cfg.XSG)
                wid = nt * 128 + extra
                wb = w_pool.tile([128, wid], U8, tag=f"wb{bi}",
                                 name=f"wb{bi}")
                nc.sync.dma_start(out=wb[:], in_=w_all[:, c0:c0 + wid])
                blocks.append((seq0 + nt, seq0 * 128, wb))
                seq0 += nt

            for bi in range(len(cfg.PLAN)):
                emit_block(bi)

            def wtile(seq):
                for s1, c0, wb in blocks:
                    if seq < s1:
                        o = seq * 128 - c0
                        return wb[:, o:o + 128].bitcast(FP8)
                raise AssertionError(seq)

            # x / scale views into block 0's extra bytes
            wb0 = blocks[0][2]
            p0 = cfg.PLAN[0] * 128
            x_t = wb0[:, p0:p0 + cfg.XB].bitcast(BF16)        # [128, KC*B]
            sgu_t = wb0[:, p0 + cfg.XB:p0 + cfg.XSG].bitcast(F32)

            # (no PE warm-up matmuls: the PE is semaphore/stream-bound,
            # and every tensor instruction costs IQ-fetch bytes on DMA
            # engine 64 — the stream's critical engine.  Keeping the
            # program just past 8x256 instructions also let the final
            # 16KB IQ refill land AFTER the last weight block, gating
            # the last down matmuls on instruction fetch.)
            h_bf = act_pool.tile([128, IT * B], BF16, tag="hbf")
            # down accumulators split across two banks so the first
            # half's PSUM->SBUF copy + DMA-out can overlap the second
            # half's final matmuls (same bank would serialize: fatal
            # PE-W + DVE-R collision otherwise)
            HT2 = HT // 2
            ps_dnA = ps_pool.tile([128, 512], F32, tag="dnA")
            ps_dnB = ps_pool.tile([128, 512], F32, tag="dnB")

            def emit_dn(j, lo, hi):
                """Down matmuls for i-tile j, output tiles [lo, hi):
                long-lived accumulation groups, one start/stop per bank."""
                base = cfg.dn_base(j)
                for ht in range(lo, hi):
                    ps = ps_dnA if ht < HT2 else ps_dnB
                    c = (ht if ht < HT2 else ht - HT2) * B
                    nc.tensor.matmul(
                        ps[:, c:c + B], wtile(base + ht),
                        h_bf[:, j * B:(j + 1) * B],
                        start=(j == 0 and ht in (0, HT2)),
                        stop=(j == IT - 1 and ht in (HT2 - 1, HT - 1)))

            for it in range(IT):
                if it == IT - 1:
                    # pad the tensor program so the 2048-instruction IQ
                    # block boundary (demand-paged, ~1.5-2.5us fetch
                    # stall on busy engine 64) lands inside gu(IT-1),
                    # where the PE is semaphore-blocked anyway, instead
                    # of in the exposed post-stream burst
                    for _ in range(96):
                        nc.tensor.nop(nofuse=True)
                # ---- gate & up for i-tile it (ping-pong PSUM banks so
                # ACT/DVE read bank parity p while PE fills parity 1-p)
                p = it % 2
                ps_g = ps_pool.tile([128, 512], F32, tag=f"g{p}")
                ps_u = ps_pool.tile([128, 512], F32, tag=f"u{p}")
                base = cfg.gu_base(it)
                for gu, ps in ((0, ps_g), (1, ps_u)):
                    for k in range(KC):
                        nc.tensor.matmul(
                            ps[:, 0:B], wtile(base + gu * KC + k),
                            x_t[:, k * B:(k + 1) * B],
                            start=(k == 0), stop=(k == KC - 1))
                # ---- SwiGLU for i-tile it:
                #   h = G*sigmoid(sg*G) * U*(sg*su)
                # with G/U the raw psum gate/up and sg/su the quant
                # scales (pad rows have zero weights => G=U=0 => h=0).
                sg_col = sgu_t[:, it * B:it * B + 1]
                sp_col = sgu_t[:, (IT + it) * B:(IT + it) * B + 1]
                sig = act_pool.tile([128, B], F32, tag=f"sig{p}")
                nc.scalar.activation(
                    out=sig[:], in_=ps_g[:, 0:B],
                    func=mybir.ActivationFunctionType.Sigmoid, scale=sg_col)
                a_sb = act_pool.tile([128, B], F32, tag=f"a{p}")
                nc.vector.tensor_mul(out=a_sb[:], in0=ps_g[:, 0:B],
                                     in1=sig[:])
                nc.vector.scalar_tensor_tensor(
                    out=h_bf[:, it * B:(it + 1) * B], in0=ps_u[:, 0:B],
                    scalar=sp_col, in1=a_sb[:],
                    op0=mybir.AluOpType.mult, op1=mybir.AluOpType.mult)
                # ---- down matmuls, two i-tiles behind (keeps the PE
                # consuming the stream in DMA arrival order)
                if it >= 2:
                    emit_dn(it - 2, 0, HT)
            # last two i-tiles; flush the A half while B's matmuls run
            y_sb = act_pool.tile([128, HT * B], F32, tag="ysb")
            if IT >= 2:
                emit_dn(IT - 2, 0, HT)
            emit_dn(IT - 1, 0, HT2)
            nc.vector.tensor_copy(out=y_sb[:, 0:HT2 * B],
                                  in_=ps_dnA[:, 0:HT2 * B])
            nc.sync.dma_start(out=y2[:, 0:HT2 * B], in_=y_sb[:, 0:HT2 * B])
            emit_dn(IT - 1, HT2, HT)
            nc.vector.tensor_copy(out=y_sb[:, HT2 * B:HT * B],
                                  in_=ps_dnB[:, 0:HT2 * B])
            nc.sync.dma_start(out=y2[:, HT2 * B:HT * B],
                              in_=y_sb[:, HT2 * B:HT * B])

    nc.compile()
    return nc


# ---------------------------------------------------------------------------
# host-side preparation
# ---------------------------------------------------------------------------

def _quant(w):
    """Reference int4 fake-quant: integer q (f32) and per-row scale."""
    w = np.asarray(w, np.float32)
    scale = (np.max(np.abs(w), axis=1, keepdims=True) /
             np.float32(7.0)).astype(np.float32)
    scale = np.maximum(scale, np.float32(np.finfo(np.float32).tiny))
    q = np.clip(np.round((w / scale).astype(np.float32)), -8.0, 7.0).astype(
        np.float32)
    return q, scale


def make_in_maps(x, w_gate, w_up, w_down, cfg):
    """Returns (in_maps for 8 cores, down-scale vector [H])."""
    B, H, IT, KC, HT = cfg.B, cfg.H, cfg.IT, cfg.KC, cfg.HT
    qg, sgf = _quant(w_gate)
    qu, suf = _quant(w_up)
    qd, sdf = _quant(w_down)

    # x: [B,1,H] f32 -> [128, KC, B] bf16  ([r,k,b] = x[b, k*128+r])
    x2 = np.asarray(x, np.float32).reshape(B, H)
    xt = np.ascontiguousarray(
        x2.T.reshape(KC, 128, B).transpose(1, 0, 2).astype(NP_BF16))

    in_maps = []
    for c in range(NCORES):
        isl = slice(c * cfg.I_SH, (c + 1) * cfg.I_SH)
        qg_sh = np.zeros((cfg.I_PAD, H), np.float32)
        qg_sh[0:cfg.I_SH] = qg[isl]
        qu_sh = np.zeros((cfg.I_PAD, H), np.float32)
        qu_sh[0:cfg.I_SH] = qu[isl]
        qd_sh = np.zeros((H, cfg.I_PAD), np.float32)
        qd_sh[:, 0:cfg.I_SH] = qd[:, isl]

        # flat-column layout in exact PE/stream consumption order:
        # gu(0), gu(1), dn(0), gu(2), dn(1), ..., gu(IT-1), dn(IT-2),
        # dn(IT-1).  gate/up tile (it,k) = [128 h-part, 128 i-cols];
        # down tile (j,ht) = [128 i-part, 128 h-cols].
        def gu_seg(it):
            out = []
            for q_sh in (qg_sh, qu_sh):
                qq = q_sh[it * 128:(it + 1) * 128]       # [128 i, H]
                out.append(qq.T.reshape(KC, 128, 128).transpose(1, 0, 2)
                           .reshape(128, KC * 128))
            return out

        def dn_seg(j):
            wd = qd_sh[:, j * 128:(j + 1) * 128].T       # [128 i, H]
            return [wd.reshape(128, HT * 128)]

        cols = []
        for it in range(IT):
            cols += gu_seg(it)
            if it >= 2:
                cols += dn_seg(it - 2)
        if IT >= 2:
            cols += dn_seg(IT - 2)
        cols += dn_seg(IT - 1)
        w_tiles = np.concatenate(cols, axis=1).astype(NP_FP8)
        assert w_tiles.shape[1] == cfg.CW

        # per-row scales, broadcast over batch: [r, it*B+b] = s[it*128+r]
        def sc_b(s_col):               # [I_SH] -> [128, IT*B] f32
            s_pad = np.zeros((cfg.I_PAD,), np.float32)
            s_pad[0:cfg.I_SH] = s_col
            return np.ascontiguousarray(np.broadcast_to(
                s_pad.reshape(IT, 128, 1).transpose(1, 0, 2),
                (128, IT, B)).reshape(128, IT * B))

        sg_sh = sgf[isl, 0]
        sp_sh = sgf[isl, 0] * suf[isl, 0]
        sgu_np = np.ascontiguousarray(np.concatenate(
            [sc_b(sg_sh), sc_b(sp_sh)], axis=1).astype(np.float32))

        # splice x + scales as raw bytes after block 0's tiles
        p0 = cfg.PLAN[0] * 128
        w_u8 = w_tiles.view(np.uint8)
        x_u8 = np.ascontiguousarray(xt.reshape(128, -1)).view(np.uint8)
        s_u8 = sgu_np.view(np.uint8)
        w_all = np.ascontiguousarray(np.concatenate(
            [w_u8[:, :p0], x_u8, s_u8, w_u8[:, p0:]], axis=1))
        assert w_all.shape[1] == cfg.CW + cfg.XSG

        in_maps.append({"w_all": w_all})
    return in_maps, sdf[:, 0]


_NC_CACHE = {}


def _get_nc(cfg):
    key = (cfg.B, cfg.H, cfg.I_FULL)
    if key not in _NC_CACHE:
        nc = bacc.Bacc(None, target_bir_lowering=False)
        build(nc, cfg)
        _NC_CACHE[key] = nc
    return _NC_CACHE[key]


def run(x, w_gate, w_up, w_down, cfg=FULL, **spmd_kwargs):
    """Full pipeline; returns (output [B,1,H] f32, BassKernelResults)."""
    in_maps, sd = make_in_maps(x, w_gate, w_up, w_down, cfg)
    nc = _get_nc(cfg)
    res = bass_utils.run_bass_kernel_spmd(
        nc, in_maps, core_ids=list(range(NCORES)), **spmd_kwargs)
    acc = np.zeros((128, cfg.HT * cfg.B), np.float32)
    for r in res.results:
        acc += r["y2"]
    # y2 [r, ht*B+b] = partial y[b, ht*128+r]
    y = acc.reshape(128, cfg.HT, cfg.B).transpose(2, 1, 0).reshape(
        cfg.B, cfg.H)
    y = y * sd[None, :]
    return y.reshape(cfg.B, 1, cfg.H).astype(np.float32), res


def kernel(x, w_gate, w_up, w_down):
    out, _ = run(x, w_gate, w_up, w_down)
    return out
